# revision 4
# baseline (speedup 1.0000x reference)
"""Trainium2 Bass kernel for MoE top-2 routing (softmax + l_aux + combine weights).

Problem: logits/mask1/mask2 [8192, 64], locations1/2 one-hot [8192, 256].
Outputs: l_aux scalar and combine_weights [8192, 64, 256].

Key structural facts exploited:
  * mask1/mask2 are disjoint one-hot rows and locations are one-hot rows, so
    combine_weights has exactly 2 nonzero elements per token out of 64*256.
    Each nonzero "row" combine_weights[s, e_k, :] equals g_k[s] * loc_k[s, :].
    We therefore only *scatter* 2 rows of 256 floats per token into the
    (pre-zeroed) output via indirect DMA instead of materializing 512 MiB.
  * g_k = num_k/(num1+num2) with num_k = sum_e exp(logits)*mask_k — the
    softmax denominator cancels, num1 = exp(rowmax), and denom >= eps always.
  * l_aux only needs column sums of gates (softmax) and mask1; both are
    computed as ones/recip-weighted PE matmuls accumulated in PSUM; each core
    emits its partial sums and the host finishes the tiny reduction while
    unsharding.

Sharding: tokens split 8 ways (1024 tokens per core); no cross-core
communication is needed on device.

Per-core layout: token s = 8*p + j lives on partition p, column group j
(contiguous DRAM rows per partition => large DMA descriptors).
"""

import numpy as np

import concourse.bass as bass
import concourse.bacc as bacc
import concourse.mybir as mybir
from concourse.tile import TileContext
from concourse import bass_utils

S, E, C = 8192, 64, 256
N_CORES = 8
S_LOC = S // N_CORES          # 1024 tokens per core
P = 128                       # partitions
J = S_LOC // P                # 8 tokens per partition
F32 = mybir.dt.float32
AX = mybir.AxisListType.X
MUL = mybir.AluOpType.mult
ADD = mybir.AluOpType.add

_CACHE = {}


def _build():
    nc = bacc.Bacc("TRN2", target_bir_lowering=False)

    logits = nc.declare_dram_parameter("logits", [S_LOC, E], F32, isOutput=False)
    m1d = nc.declare_dram_parameter("mask1", [S_LOC, E], F32, isOutput=False)
    m2d = nc.declare_dram_parameter("mask2", [S_LOC, E], F32, isOutput=False)
    l1d = nc.declare_dram_parameter("loc1", [S_LOC, C], F32, isOutput=False)
    l2d = nc.declare_dram_parameter("loc2", [S_LOC, C], F32, isOutput=False)
    cw = nc.declare_dram_parameter("cw", [S_LOC * E, C], F32, isOutput=True)
    partials = nc.declare_dram_parameter("partials", [1, 2 * E], F32, isOutput=True)

    # rowvals[p, j*E + e] = output row (s*E + e) of token s = 8p + j, expert e
    s_of = (8 * np.arange(P)[:, None, None] + np.arange(J)[None, :, None])
    rv_np = (s_of * E + np.arange(E)[None, None, :]).reshape(P, J * E).astype(np.float32)
    rv_dram = nc.inline_tensor(rv_np, "rowvals")

    with TileContext(nc) as tc:
        with (
            tc.tile_pool(name="sbuf", bufs=1) as pool,
            tc.tile_pool(name="psum", bufs=1, space="PSUM") as psum_pool,
        ):
            lt = pool.tile([P, J * E], F32)
            nc.sync.dma_start(lt[:], logits[:].rearrange("(p j) e -> p (j e)", p=P))
            m1 = pool.tile([P, J * E], F32)
            nc.sync.dma_start(m1[:], m1d[:].rearrange("(p j) e -> p (j e)", p=P))
            m2 = pool.tile([P, J * E], F32)
            nc.sync.dma_start(m2[:], m2d[:].rearrange("(p j) e -> p (j e)", p=P))
            rv = pool.tile([P, J * E], F32)
            nc.sync.dma_start(rv[:], rv_dram[:])
            # location loads split into halves so payload compute can start early
            l1 = pool.tile([P, J * C], F32)
            l2 = pool.tile([P, J * C], F32)
            l1v = l1d[:].rearrange("(p j) c -> p (j c)", p=P)
            l2v = l2d[:].rearrange("(p j) c -> p (j c)", p=P)
            H = J * C // 2
            for ld, lv in ((l1, l1v), (l2, l2v)):
                nc.sync.dma_start(ld[:, :H], lv[:, :H])
                nc.sync.dma_start(ld[:, H:], lv[:, H:])
            ones = pool.tile([P, 1], F32)
            nc.vector.memset(ones[:], 1.0)

            def v3(tile, inner):  # [P, J*inner] -> [P, J, inner]
                return tile[:].rearrange("p (j i) -> p j i", j=J)

            lt3, m13, m23 = v3(lt, E), v3(m1, E), v3(m2, E)

            # row max -> num1 = exp(max); exp(logits) without max-subtraction
            # (logits ~ N(0,1), so exp() is safe in f32)
            rmax = pool.tile([P, J], F32)
            nc.vector.reduce_max(rmax[:], lt3, axis=AX)
            num1 = pool.tile([P, J], F32)
            nc.scalar.activation(num1[:], rmax[:], mybir.ActivationFunctionType.Exp)

            # exp fused with per-token row-sum (softmax denominator)
            et = pool.tile([P, J * E], F32)
            sume = pool.tile([P, J], F32)
            for j in range(J):
                nc.scalar.activation(
                    et[:, j * E:(j + 1) * E],
                    lt[:, j * E:(j + 1) * E],
                    mybir.ActivationFunctionType.Exp,
                    accum_out=sume[:, j:j + 1],
                )
            rcp = pool.tile([P, J], F32)
            nc.vector.reciprocal(rcp[:], sume[:])

            # num2 = sum_e mask2 * exp(logits)
            p2 = pool.tile([P, J * E], F32)
            nc.vector.tensor_tensor(v3(p2, E), m23, v3(et, E), op=MUL)
            num2 = pool.tile([P, J], F32)
            nc.vector.reduce_sum(num2[:], v3(p2, E), axis=AX)

            den = pool.tile([P, J], F32)
            nc.vector.tensor_tensor(den[:], num1[:], num2[:], op=ADD)
            g1 = pool.tile([P, J], F32)   # g1 = num1 / den
            nc.vector.reciprocal(g1[:], den[:])
            g2 = pool.tile([P, J], F32)
            nc.vector.tensor_tensor(g2[:], num2[:], g1[:], op=MUL)
            nc.vector.tensor_tensor(g1[:], num1[:], g1[:], op=MUL)

            # scatter row indices ridx_k[p, j] = s*E + e_k  (exact in f32)
            ridx = []
            for k, mk in enumerate((m1, m2)):
                q = pool.tile([P, J * E], F32, tag=f"q{k}")
                nc.vector.tensor_tensor(v3(q, E), v3(mk, E), v3(rv, E), op=MUL)
                rf = pool.tile([P, J], F32, tag=f"rf{k}")
                nc.vector.reduce_sum(rf[:], v3(q, E), axis=AX)
                ri = pool.tile([P, J], mybir.dt.int32, tag=f"ri{k}")
                nc.vector.tensor_copy(ri[:], rf[:])
                ridx.append(ri)

            # payload rows r_k[p, j, :] = g_k[p, j] * loc_k[p, j, :]
            # term 1 on DVE (bulk), term 2 on ACT (scaled copies) to balance
            r1 = pool.tile([P, J * C], F32)
            g1b = g1[:].broadcast_to([P, J, C])
            nc.vector.tensor_tensor(v3(r1, C)[:, :J // 2], v3(l1, C)[:, :J // 2],
                                    g1b[:, :J // 2], op=MUL)
            nc.vector.tensor_tensor(v3(r1, C)[:, J // 2:], v3(l1, C)[:, J // 2:],
                                    g1b[:, J // 2:], op=MUL)
            r2 = pool.tile([P, J * C], F32)
            for j in range(J):
                nc.scalar.activation(
                    r2[:, j * C:(j + 1) * C],
                    l2[:, j * C:(j + 1) * C],
                    mybir.ActivationFunctionType.Copy,
                    scale=g2[:, j:j + 1],
                )

            # scatter the 2*S_LOC nonzero rows into the pre-zeroed output
            for j in range(J):
                for ri, rr in ((ridx[0], r1), (ridx[1], r2)):
                    nc.gpsimd.indirect_dma_start(
                        out=cw[:],
                        out_offset=bass.IndirectOffsetOnAxis(ap=ri[:, j:j + 1], axis=0),
                        in_=rr[:, j * C:(j + 1) * C],
                        in_offset=None,
                    )

            # l_aux partials: me_sum = sum_s gates, ce_sum = sum_s mask1
            me_ps = psum_pool.tile([1, E], F32, space="PSUM")
            for j in range(J):
                nc.tensor.matmul(me_ps[:], lhsT=rcp[:, j:j + 1],
                                 rhs=et[:, j * E:(j + 1) * E],
                                 start=(j == 0), stop=(j == J - 1))
            ce_ps = psum_pool.tile([1, E], F32, space="PSUM")
            for j in range(J):
                nc.tensor.matmul(ce_ps[:], lhsT=ones[:],
                                 rhs=m1[:, j * E:(j + 1) * E],
                                 start=(j == 0), stop=(j == J - 1))
            part_sb = pool.tile([1, 2 * E], F32)
            nc.vector.tensor_copy(part_sb[:1, :E], me_ps[:])
            nc.vector.tensor_copy(part_sb[:1, E:], ce_ps[:])
            nc.sync.dma_start(partials[:], part_sb[:])
    nc.finalize()
    return nc


def _get_nc():
    if "nc" not in _CACHE:
        _CACHE["nc"] = _build()
    return _CACHE["nc"]


def _in_maps(logits, mask1_float, mask2_float, locations1_sc, locations2_sc):
    maps = []
    for c in range(N_CORES):
        sl = slice(c * S_LOC, (c + 1) * S_LOC)
        maps.append({
            "logits": np.ascontiguousarray(logits[sl]),
            "mask1": np.ascontiguousarray(mask1_float[sl]),
            "mask2": np.ascontiguousarray(mask2_float[sl]),
            "loc1": np.ascontiguousarray(locations1_sc[sl]),
            "loc2": np.ascontiguousarray(locations2_sc[sl]),
        })
    return maps


def _install_ntff_shim():
    """The agent image's antenv lacks axon_hooks; provide it so trace=True
    can capture NTFF profiles via the libaxon ctypes path."""
    import sys
    import types

    if "antenv.axon_hooks" in sys.modules:
        return
    try:
        import antenv
        from trn_agent_boot.trn_boot import _ntff_profile_via_ctypes

        mod = types.ModuleType("antenv.axon_hooks")
        hook = _ntff_profile_via_ctypes("/opt/axon/libaxon_pjrt.so")
        mod._hook = hook
        mod.set_axon_ntff_profile_hook = lambda h: setattr(mod, "_hook", h)
        mod.get_axon_ntff_profile_hook = lambda: mod._hook
        sys.modules["antenv.axon_hooks"] = mod
        antenv.axon_hooks = mod
    except Exception:
        pass


def _run(inputs, trace=False, **kwargs):
    if trace:
        _install_ntff_shim()
    nc = _get_nc()
    maps = _in_maps(**{k: np.asarray(v) for k, v in inputs.items()})
    return bass_utils.run_bass_kernel_spmd(
        nc, maps, core_ids=list(range(N_CORES)), trace=trace, **kwargs
    )


def _assemble(results):
    # token s = 8p + j on core c  ->  global token c*S_LOC + s; cw rows are
    # already (s*E + e) within the core, so a plain reshape+concat works.
    cw = np.concatenate(
        [results[c]["cw"].reshape(S_LOC, E, C) for c in range(N_CORES)], axis=0
    )
    me_sum = np.zeros(E, np.float64)
    ce_sum = np.zeros(E, np.float64)
    for c in range(N_CORES):
        part = results[c]["partials"].reshape(2 * E)
        me_sum += part[:E]
        ce_sum += part[E:]
    l_aux = np.float32(E * np.sum(me_sum * ce_sum) / (S * S))
    return l_aux, cw


def kernel(**inputs):
    res = _run(inputs)
    return _assemble(res.results)
